# revision 3
# baseline (speedup 1.0000x reference)
"""Causal relative-position attention (music-transformer skew) on 8 TRN2 cores.

Sharding: (n, h) data+head parallel — each core gets N=2 x HC=2 head slices.
Per (n,h) pair, per 128-row q-block qi (causal, only s <= q blocks computed):
  - Gamma strip = Q_qi @ E^T  (relative logits in (q, r) coords, un-skewed)
  - Gamma -> bf16 -> DRAM strip; re-read with a sheared access pattern
    (flat stride W-1 per partition) which materializes the skew exactly
  - S = Q K^T  (+ skewed rel logits via identity-weight matmul accumulate)
  - P = exp(temp * S)   (no max subtraction: logits are O(+-6), safe in fp32)
  - causal within-block mask applied post-exp (zeroing)
  - O' = [V*w | w]^T @ P^T  per s-block (w = exp(temp*key_lengths) folds the
    additive key-length mask exactly into numerator and denominator)
  - O = O'[:64] / O'[64]  (softmax normalizer from the ones-column)
"""
import sys

if "/opt/trn_rl_repo" not in sys.path:
    sys.path.insert(0, "/opt/trn_rl_repo")

import numpy as np

import concourse.bass as bass
import concourse.mybir as mybir
import concourse.tile as tile
from concourse import bacc
from concourse.bass_utils import run_bass_kernel_spmd
from concourse.masks import make_identity

N, L, H, D = 2, 2048, 16, 64
NCORES = 8
HC = H // NCORES  # heads per core
P = 128           # partition block
T = L // P        # 16 q/s blocks
TEMP = float(D) ** -0.5
CHUNK = 512
WTMAX = L + P     # padded strip width for qi=15
f32 = mybir.dt.float32
bf16 = mybir.dt.bfloat16

_CACHE = {}


def _build():
    if "nc" in _CACHE:
        return _CACHE["nc"]

    nc = bacc.Bacc("TRN2", target_bir_lowering=False, debug=False, num_devices=NCORES)

    q_d = nc.dram_tensor("queries", [N, L, HC, D], f32, kind="ExternalInput")
    k_d = nc.dram_tensor("keys", [N, L, HC, D], f32, kind="ExternalInput")
    v_d = nc.dram_tensor("values", [N, L, HC, D], f32, kind="ExternalInput")
    e_d = nc.dram_tensor("pos_emb", [L, HC, D], f32, kind="ExternalInput")
    kl_d = nc.dram_tensor("key_lengths_add", [N, L], f32, kind="ExternalInput")
    o_d = nc.dram_tensor("out", [N, L, HC, D], f32, kind="ExternalOutput")

    # rotating DRAM scratch strips for the skew round-trip
    strips = [
        nc.dram_tensor(f"strip{i}", [P * WTMAX], bf16, kind="Internal")
        for i in range(4)
    ]

    with tile.TileContext(nc) as tc:
        with (
            tc.tile_pool(name="const", bufs=1) as cpool,
            tc.tile_pool(name="inp", bufs=1) as ipool,
            tc.tile_pool(name="tsp", bufs=2) as tpool,
            tc.tile_pool(name="work", bufs=2) as wpool,
            tc.tile_pool(name="chw", bufs=3) as chpool,
            tc.tile_pool(name="pgam", bufs=2, space="PSUM") as pgam,
            tc.tile_pool(name="ps", bufs=2, space="PSUM") as psp,
            tc.tile_pool(name="ppt", bufs=2, space="PSUM") as ppt,
            tc.tile_pool(name="po", bufs=2, space="PSUM") as po,
        ):
            ident = cpool.tile([P, P], f32)
            make_identity(nc, ident)
            ident_bf = cpool.tile([P, P], bf16)
            nc.gpsimd.tensor_copy(out=ident_bf, in_=ident)
            zpad = cpool.tile([P, P], bf16)
            nc.gpsimd.memset(zpad, 0.0)

            # ---- preload all inputs (big efficient DMAs) ----
            qall = ipool.tile([P, N, T, HC, D], f32)
            nc.sync.dma_start(
                out=qall, in_=q_d.ap().rearrange("n (t p) h d -> p n t h d", p=P)
            )
            kall = ipool.tile([P, N, T, HC, D], f32)
            nc.sync.dma_start(
                out=kall, in_=k_d.ap().rearrange("n (t p) h d -> p n t h d", p=P)
            )
            eall = ipool.tile([P, T, HC, D], f32)
            nc.sync.dma_start(
                out=eall, in_=e_d.ap().rearrange("(t p) h d -> p t h d", p=P)
            )
            vaug = ipool.tile([P, N, HC, T, D + 1], f32)
            vsrc = v_d.ap().rearrange("n (t p) h d -> n h p t d", p=P)
            for n in range(N):
                for h in range(HC):
                    nc.sync.dma_start(
                        out=vaug[:, n, h, :, :D], in_=vsrc[n, h]
                    )
            klsb = ipool.tile([P, N, T], f32)
            nc.sync.dma_start(
                out=klsb, in_=kl_d.ap().rearrange("n (t p) -> p n t", p=P)
            )

            # w = exp(temp * key_lengths_add); fold into V and the ones column
            wsb = ipool.tile([P, N, T], f32)
            nc.scalar.activation(
                out=wsb, in_=klsb, func=mybir.ActivationFunctionType.Exp, scale=TEMP
            )
            for n in range(N):
                for h in range(HC):
                    nc.vector.tensor_copy(out=vaug[:, n, h, :, D], in_=wsb[:, n, :])
                    for t in range(T):
                        nc.vector.tensor_scalar_mul(
                            out=vaug[:, n, h, t, :D],
                            in0=vaug[:, n, h, t, :D],
                            scalar1=wsb[:, n, t : t + 1],
                        )

            # ---- transpose E per head: ET[h] = [D, L] ----
            ets = []
            for h in range(HC):
                et = tpool.tile([D, L], f32, tag="et")
                for t in range(T):
                    tp = ppt.tile([P, P], f32, tag="pt")
                    nc.tensor.transpose(tp[:D, :], eall[:, t, h, :], ident)
                    nc.scalar.copy(out=et[:, t * P : (t + 1) * P], in_=tp[:D, :])
                ets.append(et)

            # ---- per-(n, h) attention ----
            it = 0
            for h in range(HC):
                et = ets[h]
                for n in range(N):
                    qt = tpool.tile([D, L], f32, tag="qt")
                    kt = tpool.tile([D, L], f32, tag="kt")
                    for t in range(T):
                        tp = ppt.tile([P, P], f32, tag="pt")
                        nc.tensor.transpose(tp[:D, :], qall[:, n, t, h, :], ident)
                        nc.scalar.copy(out=qt[:, t * P : (t + 1) * P], in_=tp[:D, :])
                        tp = ppt.tile([P, P], f32, tag="pt")
                        nc.tensor.transpose(tp[:D, :], kall[:, n, t, h, :], ident)
                        nc.scalar.copy(out=kt[:, t * P : (t + 1) * P], in_=tp[:D, :])

                    for qi in range(T):
                        W = P * (qi + 1)
                        WT = W + P
                        qtb = qt[:, qi * P : (qi + 1) * P]
                        strip = strips[it % len(strips)]
                        it += 1

                        # Gamma strip -> DRAM (bf16)
                        for rc in range(0, W, CHUNK):
                            cw = min(CHUNK, W - rc)
                            gps = pgam.tile([P, CHUNK], f32, tag="gam")
                            nc.tensor.matmul(
                                gps[:, :cw],
                                lhsT=qtb,
                                rhs=et[:, L - W + rc : L - W + rc + cw],
                                start=True,
                                stop=True,
                            )
                            gbf = chpool.tile([P, CHUNK], bf16, tag="gbf")
                            nc.scalar.copy(out=gbf[:, :cw], in_=gps[:, :cw])
                            nc.sync.dma_start(
                                out=bass.AP(strip, rc, [[WT, P], [1, cw]]),
                                in_=gbf[:, :cw],
                            )
                        # zero tail pad [W, W+P)
                        nc.sync.dma_start(
                            out=bass.AP(strip, W, [[WT, P], [1, P]]), in_=zpad
                        )

                        # sheared read: Sk[a, 128*sj + b] = strip[a*WT + 127 + a*(-1) ...]
                        skt = wpool.tile([P, L], bf16, tag="skt")
                        nc.sync.dma_start(
                            out=skt[:, :W].rearrange("p (j b) -> p j b", b=P),
                            in_=bass.AP(
                                strip, 127, [[WT - 1, P], [P, qi + 1], [1, P]]
                            ),
                        )

                        # S = Q K^T + skew(QE);  P = exp(temp*S);  transpose P
                        pts = wpool.tile([P, L], f32, tag="pts")
                        for sc in range(0, W, CHUNK):
                            cw = min(CHUNK, W - sc)
                            sps = psp.tile([P, CHUNK], f32, tag="s")
                            nc.tensor.matmul(
                                sps[:, :cw],
                                lhsT=qtb,
                                rhs=kt[:, sc : sc + cw],
                                start=True,
                                stop=False,
                            )
                            nc.tensor.matmul(
                                sps[:, :cw],
                                lhsT=ident_bf,
                                rhs=skt[:, sc : sc + cw],
                                start=False,
                                stop=True,
                            )
                            pch = chpool.tile([P, CHUNK], f32, tag="pch")
                            nc.scalar.activation(
                                out=pch[:, :cw],
                                in_=sps[:, :cw],
                                func=mybir.ActivationFunctionType.Exp,
                                scale=TEMP,
                            )
                            if sc + cw == W:
                                # diagonal block: zero strictly-upper entries
                                nc.gpsimd.affine_select(
                                    out=pch[:, cw - P : cw],
                                    in_=pch[:, cw - P : cw],
                                    compare_op=mybir.AluOpType.is_ge,
                                    fill=0.0,
                                    base=0,
                                    pattern=[[-1, P]],
                                    channel_multiplier=1,
                                )
                            for bj in range(0, cw, P):
                                ptp = ppt.tile([P, P], f32, tag="pt")
                                nc.tensor.transpose(
                                    ptp, pch[:, bj : bj + P], ident
                                )
                                nc.vector.tensor_copy(
                                    out=pts[:, sc + bj : sc + bj + P], in_=ptp
                                )

                        # O'^T[d|1, q] = sum_sj [V w | w]^T @ P^T
                        ops = po.tile([P, P], f32, tag="op")
                        for sj in range(qi + 1):
                            nc.tensor.matmul(
                                ops[: D + 1, :],
                                lhsT=vaug[:, n, h, sj, :],
                                rhs=pts[:, sj * P : (sj + 1) * P],
                                start=(sj == 0),
                                stop=(sj == qi),
                            )
                        osb = chpool.tile([D + 1, P], f32, tag="osb")
                        nc.scalar.copy(out=osb, in_=ops[: D + 1, :])
                        otp = ppt.tile([P, P], f32, tag="pt")
                        nc.tensor.matmul(
                            otp[:, : D + 1],
                            lhsT=osb,
                            rhs=ident[: D + 1, : D + 1],
                            is_transpose=True,
                            start=True,
                            stop=True,
                        )
                        rcp = chpool.tile([P, 1], f32, tag="rcp")
                        nc.vector.reciprocal(out=rcp, in_=otp[:, D : D + 1])
                        ofin = chpool.tile([P, D], f32, tag="ofin")
                        nc.vector.tensor_scalar_mul(
                            out=ofin, in0=otp[:, :D], scalar1=rcp
                        )
                        nc.sync.dma_start(
                            out=o_d.ap()[n, qi * P : (qi + 1) * P, h, :], in_=ofin
                        )

    nc.compile()
    _CACHE["nc"] = nc
    return nc


def kernel(**inputs):
    nc = _build()
    q = np.asarray(inputs["queries"], dtype=np.float32)
    k = np.asarray(inputs["keys"], dtype=np.float32)
    v = np.asarray(inputs["values"], dtype=np.float32)
    e = np.asarray(inputs["pos_emb"], dtype=np.float32)
    kl = np.asarray(inputs["key_lengths_add"], dtype=np.float32)

    in_maps = []
    for c in range(NCORES):
        hs = slice(c * HC, (c + 1) * HC)
        in_maps.append(
            {
                "queries": np.ascontiguousarray(q[:, :, hs, :]),
                "keys": np.ascontiguousarray(k[:, :, hs, :]),
                "values": np.ascontiguousarray(v[:, :, hs, :]),
                "pos_emb": np.ascontiguousarray(e[:, hs, :]),
                "key_lengths_add": np.ascontiguousarray(kl),
            }
        )
    res = run_bass_kernel_spmd(nc, in_maps, list(range(NCORES)))
    out = np.empty((N, L, H, D), dtype=np.float32)
    for c in range(NCORES):
        out[:, :, c * HC : (c + 1) * HC, :] = res.results[c]["out"]
    return out
